# revision 1
# baseline (speedup 1.0000x reference)
"""DenseCaptioner LSTM-gate kernel for 8 Trainium2 NeuronCores.

Role-split sharding (halves per-core HBM traffic vs. gate+batch-half
data parallelism):
  cores 0-3  run program VIS: visual + recurrent paths for gate g = core,
             full batch (two 128-row m-tiles)  -> partial logits [256,1024]
  cores 4-7  run program INP: input path for gate g = core-4, full batch
             -> partial logits [256,1024]
Host: logits[g] = vis_part[g] + inp_part[g] + b[g], then sigmoid/tanh gate
math and the prev_c recurrence.

The two programs are dispatched concurrently on disjoint device subsets
through a copy of concourse's PJRT runner that takes an explicit device
list (the stock one hardcodes jax.devices()[:n]).

Layout: batch-major matmuls (activation^T tiles stationary [128,128],
weight k-tiles streaming [128, 512 or 1024]). Hadamard intermediates are
PE-transposed on device (identity shipped from host: gpsimd faults here).
"""

import numpy as np

import jax
from jax.experimental.shard_map import shard_map
from jax.sharding import Mesh, PartitionSpec

import concourse.mybir as mybir
import concourse.tile as tile
from concourse import bacc, bass2jax

B, X, V, MM, VH, H1, H2, G = 256, 12000, 4096, 1024, 1024, 1024, 1024, 4
XP = 12032  # X padded to a multiple of 128 (94 k-tiles)
N_CORES = 8
MT = 2      # m-tiles (batch 256 = 2 x 128)

DT_NAME = "float32r"  # matmul dtype: "float32r" or "bfloat16"

_cache = {}


def _mm_dt():
    return getattr(mybir.dt, DT_NAME)


def _np_dt():
    return mybir.dt.np(_mm_dt())


def build_program(role):
    """role "vis": visual+recurrent paths; "inp": input path. Full batch."""
    dt = _mm_dt()
    f32 = mybir.dt.float32
    # fp32r needs rhs free dim >= 256 for full rate; bf16 allows 1024-wide
    n_chunk = 1024 if dt == mybir.dt.bfloat16 else 512

    nc = bacc.Bacc("TRN2", target_bir_lowering=False, debug=False)

    if role == "vis":
        act_specs = {"v1T": V, "v2T": V, "mT": MM, "hT": H2}
        w_specs = {"V1": V, "V2": V, "C1": VH, "C2": MM, "C3": H1,
                   "U1": H2, "U2": MM, "U3": H1}
    else:
        act_specs = {"xT": XP, "mT": MM}
        w_specs = {"W1": XP, "W2": MM, "W3": H1}

    acts_d = {
        name: nc.dram_tensor(name, [128, k // 128 * B], dt, kind="ExternalInput")
        for name, k in act_specs.items()
    }
    wt = {
        name: nc.dram_tensor(name, [k, H1], dt, kind="ExternalInput")
        for name, k in w_specs.items()
    }
    identD = nc.dram_tensor("identD", [128, 128], dt, kind="ExternalInput")
    out = nc.dram_tensor("out", [B, H2], f32, kind="ExternalOutput")

    with tile.TileContext(nc) as tc:
        with (
            tc.tile_pool(name="acts", bufs=1) as acts,
            tc.tile_pool(name="wstream", bufs=6) as wstream,
            tc.tile_pool(name="inter", bufs=1) as inter,
            tc.tile_pool(name="ps", bufs=2, space="PSUM") as ps,
        ):
            # --- resident activations, [128, ktile, mtile, batch] image ---
            def load_act(name):
                dram = acts_d[name]
                ktiles = act_specs[name] // 128
                t = acts.tile([128, ktiles * B], dt, tag=name)
                nc.sync.dma_start(t[:], dram.ap())
                return t.rearrange("p (t m b) -> p t m b", m=MT, b=128)

            act_sb = {name: load_act(name) for name in act_specs}

            ident_dt = acts.tile([128, 128], dt, tag="ident_dt")
            nc.sync.dma_start(ident_dt[:], identD.ap())

            def stream_mm(psums, act, wname):
                """psums[m][128, 1024] = act_m.T @ W for both m-tiles,
                streaming W k-tiles. act(k, m) -> lhsT [128, 128]."""
                ktiles = w_specs[wname] // 128
                w_dram = wt[wname].ap().rearrange("(t p) n -> t p n", p=128)
                for k in range(ktiles):
                    w = wstream.tile([128, H1], dt, tag="w")
                    nc.sync.dma_start(w[:], w_dram[k])
                    for mi in range(MT):
                        for n in range(0, H1, n_chunk):
                            nc.tensor.matmul(
                                psums[mi][:, n:n + n_chunk],
                                act(k, mi),
                                w[:, n:n + n_chunk],
                                start=(k == 0),
                                stop=(k == ktiles - 1),
                            )

            def hadamard_T(pa, pb):
                """qT[m] = transpose(pa[m] * pb[m]) as SBUF image
                [128, 8, 128] per m-tile; frees pa/pb psum slots."""
                qTs = []
                for mi in range(MT):
                    bounce = inter.tile([128, H1], f32, tag="bounce", bufs=2)
                    nc.vector.tensor_copy(bounce[:], pb[mi][:])
                    q = inter.tile([128, H1], dt, tag="q", bufs=2)
                    nc.vector.tensor_mul(q[:], pa[mi][:], bounce[:])
                    qT = inter.tile([128, (H1 // 128) * 128], dt, tag="qT", bufs=4)
                    qTv = qT.rearrange("p (t b) -> p t b", b=128)
                    for j in range(H1 // 128):
                        ptr = ps.tile([128, 128], dt, tag="s1")
                        nc.tensor.transpose(
                            ptr[:], q[:, j * 128:(j + 1) * 128], ident_dt[:]
                        )
                        nc.vector.tensor_copy(qTv[:, j, :], ptr[:])
                    qTs.append(qTv)
                return qTs

            acc = [inter.tile([128, H2], f32, tag="acc", name=f"acc{i}", bufs=2) for i in range(MT)]

            def level23(qT_in, w_m, w_out, first, lvl2_w=None):
                """acc[m] (+)= ((qT_in[@lvl2_w]) * (m @ w_m)) @ w_out."""
                src = qT_in
                if lvl2_w is not None:
                    pa2 = [ps.tile([128, H1], f32, tag="s1", name=f"pa2_{i}") for i in range(MT)]
                    stream_mm(pa2, lambda k, mi: qT_in[mi][:, k, :], lvl2_w)
                    pb2 = [ps.tile([128, H1], f32, tag="s2", name=f"pb2_{i}") for i in range(MT)]
                    stream_mm(
                        pb2, lambda k, mi: act_sb["mT"][:, k, mi, :], w_m
                    )
                    src = hadamard_T(pa2, pb2)
                l3 = [ps.tile([128, H2], f32, tag="s2", name=f"l3_{i}") for i in range(MT)]
                stream_mm(l3, lambda k, mi: src[mi][:, k, :], w_out)
                for mi in range(MT):
                    if first:
                        nc.vector.tensor_copy(acc[mi][:], l3[mi][:])
                    else:
                        nc.vector.tensor_add(acc[mi][:], acc[mi][:], l3[mi][:])

            def level1(a_name, b_name, w_a, w_b):
                pa = [ps.tile([128, H1], f32, tag="s1", name=f"pa_{i}") for i in range(MT)]
                stream_mm(pa, lambda k, mi: act_sb[a_name][:, k, mi, :], w_a)
                pb = [ps.tile([128, H1], f32, tag="s2", name=f"pb_{i}") for i in range(MT)]
                stream_mm(pb, lambda k, mi: act_sb[b_name][:, k, mi, :], w_b)
                return hadamard_T(pa, pb)

            if role == "vis":
                t1T = level1("v1T", "v2T", "V1", "V2")
                level23(t1T, "C2", "C3", first=True, lvl2_w="C1")
                hqT = level1("hT", "mT", "U1", "U2")
                level23(hqT, None, "U3", first=False)
            else:
                xqT = level1("xT", "mT", "W1", "W2")
                level23(xqT, None, "W3", first=True)

            out_v = out.ap().rearrange("(m p) n -> m p n", p=128)
            for mi in range(MT):
                nc.sync.dma_start(out_v[mi], acc[mi][:])

    nc.compile()
    return nc


def _make_runner(nc, devices):
    """Adapted from concourse.bass2jax.run_bass_via_pjrt: same lowering,
    but runs on an explicit device subset and returns unmaterialized jax
    arrays so two programs can be dispatched concurrently."""
    bass2jax.install_neuronx_cc_hook()

    assert nc.dbg_addr is None
    partition_name = (
        nc.partition_id_tensor.name if nc.partition_id_tensor else None
    )

    in_names, out_names, out_avals, zero_outs = [], [], [], []
    for alloc in nc.m.functions[0].allocations:
        if not isinstance(alloc, mybir.MemoryLocationSet):
            continue
        name = alloc.memorylocations[0].name
        if alloc.kind == "ExternalInput":
            if name != partition_name:
                in_names.append(name)
        elif alloc.kind == "ExternalOutput":
            shape = tuple(alloc.tensor_shape)
            dtype = mybir.dt.np(alloc.dtype)
            out_names.append(name)
            out_avals.append(jax.core.ShapedArray(shape, dtype))
            zero_outs.append(np.zeros(shape, dtype))
    n_params = len(in_names)
    n_outs = len(out_avals)
    in_names.extend(out_names)
    if partition_name is not None:
        in_names.append(partition_name)
    donate = tuple(range(n_params, n_params + n_outs))

    def _body(*args):
        operands = list(args)
        if partition_name is not None:
            operands.append(bass2jax.partition_id_tensor())
        outs = bass2jax._bass_exec_p.bind(
            *operands,
            out_avals=tuple(out_avals),
            in_names=tuple(in_names),
            out_names=tuple(out_names),
            lowering_input_output_aliases=(),
            sim_require_finite=True,
            sim_require_nnan=True,
            nc=nc,
        )
        return tuple(outs)

    n_cores = len(devices)
    mesh = Mesh(np.asarray(devices), ("core",))
    in_specs = (PartitionSpec("core"),) * (n_params + n_outs)
    out_specs = (PartitionSpec("core"),) * n_outs
    sharded = jax.jit(
        shard_map(
            _body, mesh=mesh, in_specs=in_specs, out_specs=out_specs,
            check_rep=False,
        ),
        donate_argnums=donate,
        keep_unused=True,
    )

    def run(in_maps):
        assert len(in_maps) == n_cores
        concat_in = [
            np.concatenate(
                [np.asarray(in_maps[c][name]) for c in range(n_cores)], axis=0
            )
            for name in in_names[:n_params]
        ]
        concat_zeros = [
            np.zeros((n_cores * z.shape[0], *z.shape[1:]), z.dtype)
            for z in zero_outs
        ]
        out_arrs = sharded(*concat_in, *concat_zeros)
        return out_names, out_avals, out_arrs

    return run


def _tile_actT(a, kdim):
    """[256 batch, K<=kdim] -> SBUF image [128, (kdim/128) * 256]:
    (p, (t*2+mi)*128+b) = a[mi*128+b, t*128+p], contiguous per partition."""
    ktiles = kdim // 128
    a = np.asarray(a, np.float32)
    if a.shape[1] < kdim:
        a = np.pad(a, ((0, 0), (0, kdim - a.shape[1])))
    # [2m, 128b, ktiles, 128p] -> [128p, ktiles, 2m, 128b]
    r = a.reshape(MT, 128, ktiles, 128).transpose(3, 2, 0, 1)
    return np.ascontiguousarray(r.reshape(128, ktiles * B), dtype=_np_dt())


def kernel(prev_h, prev_c, x, m, v1, v2, V1, V2, C1, C2, C3, W1, W2, W3, U1, U2, U3, b):
    npdt = _np_dt()
    if "runners" not in _cache:
        devs = jax.devices()
        nc_vis = build_program("vis")
        nc_inp = build_program("inp")
        _cache["runners"] = (
            _make_runner(nc_vis, devs[0:4]),
            _make_runner(nc_inp, devs[4:8]),
        )
        _cache["ncs"] = (nc_vis, nc_inp)
    run_vis, run_inp = _cache["runners"]

    ident = np.eye(128, dtype=np.float32).astype(npdt)

    v1T_img = _tile_actT(v1, V)
    v2T_img = _tile_actT(v2, V)
    mT_img = _tile_actT(m, MM)
    hT_img = _tile_actT(prev_h, H2)
    xT_img = _tile_actT(x, XP)

    vis_maps, inp_maps = [], []
    for g in range(G):
        vis_maps.append({
            "v1T": v1T_img, "v2T": v2T_img, "mT": mT_img, "hT": hT_img,
            "V1": np.ascontiguousarray(V1[g], dtype=npdt),
            "V2": np.ascontiguousarray(V2[g], dtype=npdt),
            "C1": np.ascontiguousarray(C1[g], dtype=npdt),
            "C2": np.ascontiguousarray(C2[g], dtype=npdt),
            "C3": np.ascontiguousarray(C3[g], dtype=npdt),
            "U1": np.ascontiguousarray(U1[g], dtype=npdt),
            "U2": np.ascontiguousarray(U2[g], dtype=npdt),
            "U3": np.ascontiguousarray(U3[g], dtype=npdt),
            "identD": ident,
        })
        w1_pad = np.zeros((XP, H1), np.float32)
        w1_pad[:X] = np.asarray(W1[g], np.float32)
        inp_maps.append({
            "xT": xT_img, "mT": mT_img,
            "W1": np.ascontiguousarray(w1_pad, dtype=npdt),
            "W2": np.ascontiguousarray(W2[g], dtype=npdt),
            "W3": np.ascontiguousarray(W3[g], dtype=npdt),
            "identD": ident,
        })

    _cache["last_in_maps"] = (vis_maps, inp_maps)

    # dispatch both programs; they run concurrently on disjoint cores
    vnames, vavals, vouts = run_vis(vis_maps)
    inames, iavals, iouts = run_inp(inp_maps)

    vis_out = np.asarray(vouts[0]).reshape(G, B, H2)
    inp_out = np.asarray(iouts[0]).reshape(G, B, H2)

    logits = vis_out + inp_out + np.asarray(b, np.float32)[:, None, :]

    def sigmoid(z):
        return 1.0 / (1.0 + np.exp(-z))

    i = sigmoid(logits[0])
    f = sigmoid(logits[1])
    o = sigmoid(logits[2])
    cg = np.tanh(logits[3])
    prev_c = np.asarray(prev_c, np.float32)
    new_c = f * prev_c + i * cg
    new_h = o * np.tanh(prev_c)
    return new_h.astype(np.float32), new_c.astype(np.float32)



# revision 3
# speedup vs baseline: 1.4583x; 1.4583x over previous
"""DenseCaptioner LSTM-gate kernel for 8 Trainium2 NeuronCores.

Role-split sharding (halves per-core HBM traffic vs. gate+batch-half
data parallelism):
  cores 0-3  run program VIS: visual + recurrent paths for gate g = core,
             full batch (two 128-row m-tiles)  -> partial logits [256,1024]
  cores 4-7  run program INP: input path for gate g = core-4, full batch
             -> partial logits [256,1024]
Host: logits[g] = vis_part[g] + inp_part[g] + b[g], then sigmoid/tanh gate
math and the prev_c recurrence.

The two programs are dispatched concurrently on disjoint device subsets
through a copy of concourse's PJRT runner that takes an explicit device
list (the stock one hardcodes jax.devices()[:n]).

Layout: batch-major matmuls (activation^T tiles stationary [128,128],
weight k-tiles streaming [128, 512 or 1024]). Hadamard intermediates are
PE-transposed on device (identity shipped from host: gpsimd faults here).
"""

import numpy as np

import jax
from jax.experimental.shard_map import shard_map
from jax.sharding import Mesh, PartitionSpec

import concourse.mybir as mybir
import concourse.tile as tile
from concourse import bacc, bass2jax

B, X, V, MM, VH, H1, H2, G = 256, 12000, 4096, 1024, 1024, 1024, 1024, 4
XP = 12032  # X padded to a multiple of 128 (94 k-tiles)
N_CORES = 8
MT = 2      # m-tiles (batch 256 = 2 x 128)

DT_NAME = "bfloat16"  # matmul dtype: "float32r" or "bfloat16"

_cache = {}


def _mm_dt():
    return getattr(mybir.dt, DT_NAME)


def _np_dt():
    return mybir.dt.np(_mm_dt())


def build_program(role):
    """role "vis": visual+recurrent paths; "inp": input path. Full batch."""
    dt = _mm_dt()
    f32 = mybir.dt.float32
    # max matmul free dim is 512 (one PSUM bank); >=256 needed for full rate
    n_chunk = 512

    nc = bacc.Bacc("TRN2", target_bir_lowering=False, debug=False)

    if role == "vis":
        act_specs = {"v1T": V, "v2T": V, "mT": MM, "hT": H2}
        w_specs = {"V1": V, "V2": V, "C1": VH, "C2": MM, "C3": H1,
                   "U1": H2, "U2": MM, "U3": H1}
    else:
        act_specs = {"xT": XP, "mT": MM}
        w_specs = {"W1": XP, "W2": MM, "W3": H1}

    acts_d = {
        name: nc.dram_tensor(name, [128, k // 128 * B], dt, kind="ExternalInput")
        for name, k in act_specs.items()
    }
    wt = {
        name: nc.dram_tensor(name, [k, H1], dt, kind="ExternalInput")
        for name, k in w_specs.items()
    }
    identD = nc.dram_tensor("identD", [128, 128], dt, kind="ExternalInput")
    out = nc.dram_tensor("out", [B, H2], f32, kind="ExternalOutput")

    with tile.TileContext(nc) as tc:
        with (
            tc.tile_pool(name="acts", bufs=1) as acts,
            tc.tile_pool(name="wstream", bufs=6) as wstream,
            tc.tile_pool(name="inter", bufs=1) as inter,
            tc.tile_pool(name="ps", bufs=2, space="PSUM") as ps,
        ):
            # --- resident activations, [128, ktile, mtile, batch] image ---
            def load_act(name):
                dram = acts_d[name]
                ktiles = act_specs[name] // 128
                t = acts.tile([128, ktiles * B], dt, tag=name)
                nc.sync.dma_start(t[:], dram.ap())
                return t.rearrange("p (t m b) -> p t m b", m=MT, b=128)

            act_sb = {name: load_act(name) for name in act_specs}

            ident_dt = acts.tile([128, 128], dt, tag="ident_dt")
            nc.sync.dma_start(ident_dt[:], identD.ap())

            def stream_mm(psums, act, wname):
                """psums[m][128, 1024] = act_m.T @ W for both m-tiles,
                streaming W k-tiles. act(k, m) -> lhsT [128, 128]."""
                ktiles = w_specs[wname] // 128
                w_dram = wt[wname].ap().rearrange("(t p) n -> t p n", p=128)
                for k in range(ktiles):
                    w = wstream.tile([128, H1], dt, tag="w")
                    nc.sync.dma_start(w[:], w_dram[k])
                    for mi in range(MT):
                        for n in range(0, H1, n_chunk):
                            nc.tensor.matmul(
                                psums[mi][:, n:n + n_chunk],
                                act(k, mi),
                                w[:, n:n + n_chunk],
                                start=(k == 0),
                                stop=(k == ktiles - 1),
                            )

            def hadamard_T(pa, pb):
                """qT[m] = transpose(pa[m] * pb[m]) as SBUF image
                [128, 8, 128] per m-tile; frees pa/pb psum slots."""
                qTs = []
                for mi in range(MT):
                    bounce = inter.tile([128, H1], f32, tag="bounce", bufs=2)
                    nc.vector.tensor_copy(bounce[:], pb[mi][:])
                    q = inter.tile([128, H1], dt, tag="q", bufs=2)
                    nc.vector.tensor_mul(q[:], pa[mi][:], bounce[:])
                    qT = inter.tile([128, (H1 // 128) * 128], dt, tag="qT", bufs=4)
                    qTv = qT.rearrange("p (t b) -> p t b", b=128)
                    for j in range(H1 // 128):
                        ptr = ps.tile([128, 128], dt, tag="s1")
                        nc.tensor.transpose(
                            ptr[:], q[:, j * 128:(j + 1) * 128], ident_dt[:]
                        )
                        nc.vector.tensor_copy(qTv[:, j, :], ptr[:])
                    qTs.append(qTv)
                return qTs

            acc = [inter.tile([128, H2], f32, tag="acc", name=f"acc{i}", bufs=2) for i in range(MT)]

            def level23(qT_in, w_m, w_out, first, lvl2_w=None):
                """acc[m] (+)= ((qT_in[@lvl2_w]) * (m @ w_m)) @ w_out."""
                src = qT_in
                if lvl2_w is not None:
                    pa2 = [ps.tile([128, H1], f32, tag="s1", name=f"pa2_{i}") for i in range(MT)]
                    stream_mm(pa2, lambda k, mi: qT_in[mi][:, k, :], lvl2_w)
                    pb2 = [ps.tile([128, H1], f32, tag="s2", name=f"pb2_{i}") for i in range(MT)]
                    stream_mm(
                        pb2, lambda k, mi: act_sb["mT"][:, k, mi, :], w_m
                    )
                    src = hadamard_T(pa2, pb2)
                l3 = [ps.tile([128, H2], f32, tag="s2", name=f"l3_{i}") for i in range(MT)]
                stream_mm(l3, lambda k, mi: src[mi][:, k, :], w_out)
                for mi in range(MT):
                    if first:
                        nc.vector.tensor_copy(acc[mi][:], l3[mi][:])
                    else:
                        nc.vector.tensor_add(acc[mi][:], acc[mi][:], l3[mi][:])

            def level1(a_name, b_name, w_a, w_b):
                pa = [ps.tile([128, H1], f32, tag="s1", name=f"pa_{i}") for i in range(MT)]
                stream_mm(pa, lambda k, mi: act_sb[a_name][:, k, mi, :], w_a)
                pb = [ps.tile([128, H1], f32, tag="s2", name=f"pb_{i}") for i in range(MT)]
                stream_mm(pb, lambda k, mi: act_sb[b_name][:, k, mi, :], w_b)
                return hadamard_T(pa, pb)

            if role == "vis":
                t1T = level1("v1T", "v2T", "V1", "V2")
                level23(t1T, "C2", "C3", first=True, lvl2_w="C1")
                hqT = level1("hT", "mT", "U1", "U2")
                level23(hqT, None, "U3", first=False)
            else:
                xqT = level1("xT", "mT", "W1", "W2")
                level23(xqT, None, "W3", first=True)

            out_v = out.ap().rearrange("(m p) n -> m p n", p=128)
            for mi in range(MT):
                nc.sync.dma_start(out_v[mi], acc[mi][:])

    nc.compile()
    return nc


def _make_runner(nc, devices):
    """Adapted from concourse.bass2jax.run_bass_via_pjrt: same lowering,
    but runs on an explicit device subset and returns unmaterialized jax
    arrays so two programs can be dispatched concurrently."""
    bass2jax.install_neuronx_cc_hook()

    assert nc.dbg_addr is None
    partition_name = (
        nc.partition_id_tensor.name if nc.partition_id_tensor else None
    )

    in_names, out_names, out_avals, zero_outs = [], [], [], []
    for alloc in nc.m.functions[0].allocations:
        if not isinstance(alloc, mybir.MemoryLocationSet):
            continue
        name = alloc.memorylocations[0].name
        if alloc.kind == "ExternalInput":
            if name != partition_name:
                in_names.append(name)
        elif alloc.kind == "ExternalOutput":
            shape = tuple(alloc.tensor_shape)
            dtype = mybir.dt.np(alloc.dtype)
            out_names.append(name)
            out_avals.append(jax.core.ShapedArray(shape, dtype))
            zero_outs.append(np.zeros(shape, dtype))
    n_params = len(in_names)
    n_outs = len(out_avals)
    in_names.extend(out_names)
    if partition_name is not None:
        in_names.append(partition_name)
    donate = tuple(range(n_params, n_params + n_outs))

    def _body(*args):
        operands = list(args)
        if partition_name is not None:
            operands.append(bass2jax.partition_id_tensor())
        outs = bass2jax._bass_exec_p.bind(
            *operands,
            out_avals=tuple(out_avals),
            in_names=tuple(in_names),
            out_names=tuple(out_names),
            lowering_input_output_aliases=(),
            sim_require_finite=True,
            sim_require_nnan=True,
            nc=nc,
        )
        return tuple(outs)

    n_cores = len(devices)
    mesh = Mesh(np.asarray(devices), ("core",))
    in_specs = (PartitionSpec("core"),) * (n_params + n_outs)
    out_specs = (PartitionSpec("core"),) * n_outs
    sharded = jax.jit(
        shard_map(
            _body, mesh=mesh, in_specs=in_specs, out_specs=out_specs,
            check_rep=False,
        ),
        donate_argnums=donate,
        keep_unused=True,
    )

    def run(in_maps):
        assert len(in_maps) == n_cores
        concat_in = [
            np.concatenate(
                [np.asarray(in_maps[c][name]) for c in range(n_cores)], axis=0
            )
            for name in in_names[:n_params]
        ]
        concat_zeros = [
            np.zeros((n_cores * z.shape[0], *z.shape[1:]), z.dtype)
            for z in zero_outs
        ]
        out_arrs = sharded(*concat_in, *concat_zeros)
        return out_names, out_avals, out_arrs

    return run


def _tile_actT(a, kdim):
    """[256 batch, K<=kdim] -> SBUF image [128, (kdim/128) * 256]:
    (p, (t*2+mi)*128+b) = a[mi*128+b, t*128+p], contiguous per partition."""
    ktiles = kdim // 128
    a = np.asarray(a, np.float32)
    if a.shape[1] < kdim:
        a = np.pad(a, ((0, 0), (0, kdim - a.shape[1])))
    # [2m, 128b, ktiles, 128p] -> [128p, ktiles, 2m, 128b]
    r = a.reshape(MT, 128, ktiles, 128).transpose(3, 2, 0, 1)
    return np.ascontiguousarray(r.reshape(128, ktiles * B), dtype=_np_dt())


def kernel(prev_h, prev_c, x, m, v1, v2, V1, V2, C1, C2, C3, W1, W2, W3, U1, U2, U3, b):
    npdt = _np_dt()
    if "runners" not in _cache:
        devs = jax.devices()
        nc_vis = build_program("vis")
        nc_inp = build_program("inp")
        _cache["runners"] = (
            _make_runner(nc_vis, devs[0:4]),
            _make_runner(nc_inp, devs[4:8]),
        )
        _cache["ncs"] = (nc_vis, nc_inp)
    run_vis, run_inp = _cache["runners"]

    ident = np.eye(128, dtype=np.float32).astype(npdt)

    v1T_img = _tile_actT(v1, V)
    v2T_img = _tile_actT(v2, V)
    mT_img = _tile_actT(m, MM)
    hT_img = _tile_actT(prev_h, H2)
    xT_img = _tile_actT(x, XP)

    vis_maps, inp_maps = [], []
    for g in range(G):
        vis_maps.append({
            "v1T": v1T_img, "v2T": v2T_img, "mT": mT_img, "hT": hT_img,
            "V1": np.ascontiguousarray(V1[g], dtype=npdt),
            "V2": np.ascontiguousarray(V2[g], dtype=npdt),
            "C1": np.ascontiguousarray(C1[g], dtype=npdt),
            "C2": np.ascontiguousarray(C2[g], dtype=npdt),
            "C3": np.ascontiguousarray(C3[g], dtype=npdt),
            "U1": np.ascontiguousarray(U1[g], dtype=npdt),
            "U2": np.ascontiguousarray(U2[g], dtype=npdt),
            "U3": np.ascontiguousarray(U3[g], dtype=npdt),
            "identD": ident,
        })
        w1_pad = np.zeros((XP, H1), np.float32)
        w1_pad[:X] = np.asarray(W1[g], np.float32)
        inp_maps.append({
            "xT": xT_img, "mT": mT_img,
            "W1": np.ascontiguousarray(w1_pad, dtype=npdt),
            "W2": np.ascontiguousarray(W2[g], dtype=npdt),
            "W3": np.ascontiguousarray(W3[g], dtype=npdt),
            "identD": ident,
        })

    _cache["last_in_maps"] = (vis_maps, inp_maps)

    # dispatch both programs; they run concurrently on disjoint cores
    vnames, vavals, vouts = run_vis(vis_maps)
    inames, iavals, iouts = run_inp(inp_maps)

    vis_out = np.asarray(vouts[0]).reshape(G, B, H2)
    inp_out = np.asarray(iouts[0]).reshape(G, B, H2)

    logits = vis_out + inp_out + np.asarray(b, np.float32)[:, None, :]

    def sigmoid(z):
        return 1.0 / (1.0 + np.exp(-z))

    i = sigmoid(logits[0])
    f = sigmoid(logits[1])
    o = sigmoid(logits[2])
    cg = np.tanh(logits[3])
    prev_c = np.asarray(prev_c, np.float32)
    new_c = f * prev_c + i * cg
    new_h = o * np.tanh(prev_c)
    return new_h.astype(np.float32), new_c.astype(np.float32)

